# revision 9
# baseline (speedup 1.0000x reference)
"""Trainium2 Bass kernel: 3D 'same' convolution (implicit GEMM).

Problem: x (4, 64, 24, 24, 24) f32, weight (1, 128, 1728) f32
         -> out (4, 128, 24, 24, 24) f32  (SAME conv3d, k=3)

Sharding (8 cores): batch (4) x z-halves (2). Each core computes
out[b, :, z0:z0+12] for its (b, zh) shard; no inter-core communication.

Per-core algorithm: 27-tap implicit GEMM in bf16, with taps processed
two-at-a-time per matmul at full K=128 contraction. PE throughput is
set by the single rhs-streaming XBUS (1 column/cycle total), so the
goal is min(total streamed columns) = ceil(27/2) passes x 6912 output
positions. Packing two taps per pass needs the top 64 partitions to
hold the input window *shifted by the tap delta*: three SBUF tiles
carry replicas shifted by +1x, +1y, +1z. 27 taps = 9 x-pairs (tile A)
+ 3 y-pairs (tile B) + 1 z-pair (tile C) + 1 single (K=64).

Full-row (K=128) matmuls cannot overlap their LDWEIGHTS with the
previous matmul (row groups always conflict), so each weight load is
reused across a group of 7 output tiles (7 PSUM banks in flight,
pass-outer loop) and the redundant legalization-inserted LDWEIGHTS
are elided post-finalize: 196 loads -> ~28.

Output tiles are one z-plane x 21 y-rows x 24 (N=504); the y=21..23
remainder rows are batched across 6 z-planes (N=432) per half-shard.
Group 1 = planes z=0..5 + rem(0..5) reads only padded planes 0..8;
group 2 = planes 6..11 + rem(6..11) reads planes 6..13 — X is loaded
as two overlapping z-chunks so next iteration's chunk-1 DMA overlaps
this iteration's group-2 compute. ACT evacuates PSUM->SBUF with
fp32->bf16 downcast; outputs are stored bf16 and upcast on host
(total quantization error ~3e-3 rel, tolerance 2e-2).
"""

import sys

if "/opt/trn_rl_repo" not in sys.path:
    sys.path.insert(0, "/opt/trn_rl_repo")

import ml_dtypes
import numpy as np

CIN, COUT, K = 64, 128, 3
DHW = 24  # cubic spatial extent
ZS = 12  # z-planes per shard
NP = 14  # padded z-planes per shard window (ZS + 2 halo)
ZA, ZB0, ZB = 9, 6, 8  # chunk1 planes 0..8, chunk2 planes 6..13
PW = 26  # padded y/x extent
N_CORES = 8

# pass table: (tile, tapA, tapB) with tapB = tapA + tile's shift delta;
# tapB None -> single K=64 pass on the tile's bottom half.
# tiles: 0=A (shift +1x), 1=B (+1y), 2=C (+1z)
PASSES = (
    [(0, (dz, dy, 0), (dz, dy, 1)) for dz in range(3) for dy in range(3)]
    + [(1, (dz, 0, 2), (dz, 1, 2)) for dz in range(3)]
    + [(2, (0, 2, 2), (1, 2, 2))]
    + [(0, (2, 2, 2), None)]
)
NPASS = len(PASSES)  # 14


def _elide_redundant_ldweights(nc):
    """Drop legalization-inserted LDWEIGHTS that reload the identical
    weights AP already resident in the PE array (same block, no
    intervening different load). Only sync-free loads are elided."""
    n_drop = 0
    for f in nc.m.functions:
        for b in f.blocks:
            last_key = None
            drop = []
            for inst in b.instructions:
                tn = type(inst).__name__
                if tn == "InstLdweights":
                    key = (str(inst.ins[0]), str(inst.perf_mode), str(inst.is_transpose))
                    si = inst.sync_info
                    clean = si is None or (len(si.on_wait) == 0 and len(si.on_update) == 0)
                    if key == last_key and clean:
                        drop.append(inst)
                    else:
                        last_key = key
            for inst in drop:
                b.instructions.remove(inst)
            n_drop += len(drop)
    return n_drop


def _build_program(loop_n=None, unroll=False):
    """Build the SPMD Bass program (one NeuronCore's view).

    loop_n: if set, wrap the whole body in a hardware For_i loop with
    that many iterations (used by test.py for wall-clock timing).
    unroll: python-unroll the loop instead (for TimelineSim, which
    can't follow register-mode branches).
    """
    import concourse.tile as tile
    from concourse import bacc, mybir

    BF16 = mybir.dt.bfloat16
    F32 = mybir.dt.float32

    nc = bacc.Bacc("TRN2")
    # X inputs: 3 shift-variants x 2 overlapping z-chunks
    xins = []
    for s in "abc":
        x1 = nc.declare_dram_parameter(f"x{s}1", [128, ZA, PW, PW], BF16, isOutput=False)
        x2 = nc.declare_dram_parameter(f"x{s}2", [128, ZB, PW, PW], BF16, isOutput=False)
        xins.append((x1, x2))
    wk_in = nc.declare_dram_parameter("wk", [128, NPASS, 128], BF16, isOutput=False)
    y_out = nc.declare_dram_parameter("y", [128, ZS, DHW, DHW], BF16, isOutput=True)

    with tile.TileContext(nc) as tc:
        with (
            tc.tile_pool(name="xw", bufs=1) as xw_pool,
            tc.tile_pool(name="ps", bufs=8, space="PSUM") as ps_pool,
            tc.tile_pool(name="ob", bufs=4) as ob_pool,
        ):

            def body(_iv=None):
                # two W tiles: each half's reload (next iteration) hides
                # behind the other half's matmuls instead of stalling
                Wa = xw_pool.tile([128, 7, 128], BF16, name="Wa", tag="Wa")
                nc.sync.dma_start(out=Wa[:], in_=wk_in[:, 0:7])
                Wb = xw_pool.tile([128, 7, 128], BF16, name="Wb", tag="Wb")
                nc.sync.dma_start(out=Wb[:], in_=wk_in[:, 7:14])
                XT1, XT2 = [], []
                for s, (x1, x2) in zip("abc", xins):
                    t1 = xw_pool.tile([128, ZA, PW, PW], BF16, name=f"X{s}1", tag=f"X{s}1")
                    nc.sync.dma_start(out=t1[:], in_=x1[:])
                    XT1.append(t1)
                for s, (x1, x2) in zip("abc", xins):
                    t2 = xw_pool.tile([128, ZB, PW, PW], BF16, name=f"X{s}2", tag=f"X{s}2")
                    nc.sync.dma_start(out=t2[:], in_=x2[:])
                    XT2.append(t2)

                # output tiles: ("plane", z) N=504 (21x24, 2D AP)
                #            or ("rem", zoff) N=432 (6x3x24, 3D)
                # group 1 reads padded planes 0..8 (chunk 1), group 2
                # reads planes 6..13 (chunk 2, stored from plane 6).
                groups = [
                    (XT1, 0, [("plane", z) for z in range(6)] + [("rem", 0)]),
                    (XT2, 6, [("plane", z) for z in range(6, 12)] + [("rem", 6)]),
                ]

                def rhs_ap(X, zbase, kind, zi, dz, dy, dx, lo, hi):
                    if kind == "plane":
                        return X[lo:hi, zi - zbase + dz, dy : dy + 21, dx : dx + 24]
                    z0 = zi - zbase + dz
                    return X[lo:hi, z0 : z0 + 6, 21 + dy : 24 + dy, dx : dx + 24]

                for XT, zbase, gtiles in groups:
                    pss = []
                    for kind, zi in gtiles:
                        ps = ps_pool.tile([128, 512], F32, name="ps", tag="ps")
                        pss.append(ps)
                    for j, (ti, ta, tb) in enumerate(PASSES):
                        dz, dy, dx = ta
                        lo, hi = (0, 128) if tb is not None else (0, 64)
                        for (kind, zi), ps in zip(gtiles, pss):
                            n = 504 if kind == "plane" else 432
                            nc.tensor.matmul(
                                ps[:, :n],
                                lhsT=(Wa if j < 7 else Wb)[lo:hi, j % 7, :],
                                rhs=rhs_ap(XT[ti], zbase, kind, zi, dz, dy, dx, lo, hi),
                                start=(j == 0),
                                stop=(j == NPASS - 1),
                                skip_group_check=True,
                            )
                    for (kind, zi), ps in zip(gtiles, pss):
                        n = 504 if kind == "plane" else 432
                        ob = ob_pool.tile([128, 512], BF16, name="ob", tag="ob")
                        nc.scalar.copy(ob[:, :n], ps[:, :n])
                        if kind == "plane":
                            nc.sync.dma_start(out=y_out[:, zi, 0:21, :], in_=ob[:, :n])
                        else:
                            # one DMA per z-plane: keeps each transfer one
                            # contiguous run per partition (descriptor-lean)
                            for j in range(6):
                                nc.sync.dma_start(
                                    out=y_out[:, zi + j, 21:24, :],
                                    in_=ob[:, j * 72 : (j + 1) * 72],
                                )

            if loop_n is not None:
                if unroll:
                    for _k in range(loop_n):
                        body()
                else:
                    with tc.For_i(0, loop_n, 1) as _i:
                        body(_i)
            else:
                body()

    nc.finalize()
    _elide_redundant_ldweights(nc)
    return nc


def _make_in_maps(x, weight):
    w = np.asarray(weight, np.float32).reshape(COUT, CIN, K, K, K)
    wk = np.zeros((128, NPASS, 128), np.float32)
    for j, (_ti, ta, tb) in enumerate(PASSES):
        wk[0:64, j, :] = w[:, :, ta[0], ta[1], ta[2]].T
        if tb is not None:
            wk[64:128, j, :] = w[:, :, tb[0], tb[1], tb[2]].T
    wk = wk.astype(ml_dtypes.bfloat16)

    x = np.asarray(x, np.float32)
    in_maps = []
    for c in range(N_CORES):
        b, zh = divmod(c, 2)
        z0 = zh * ZS
        xpad = np.zeros((CIN, PW, PW, PW), np.float32)
        xpad[:, 1:25, 1:25, 1:25] = x[b]
        win = xpad[:, z0 : z0 + NP]  # (64, 14, 26, 26)

        def repl(shift_axis):
            X = np.zeros((128, NP, PW, PW), np.float32)
            X[0:64] = win
            if shift_axis == 2:  # +1x
                X[64:128, :, :, :-1] = win[:, :, :, 1:]
            elif shift_axis == 1:  # +1y
                X[64:128, :, :-1, :] = win[:, :, 1:, :]
            else:  # +1z
                X[64:128, :-1] = win[:, 1:]
            return X.astype(ml_dtypes.bfloat16)

        m = {"wk": wk}
        for s, ax in (("a", 2), ("b", 1), ("c", 0)):
            X = repl(ax)
            m[f"x{s}1"] = np.ascontiguousarray(X[:, 0:ZA])
            m[f"x{s}2"] = np.ascontiguousarray(X[:, ZB0 : ZB0 + ZB])
        in_maps.append(m)
    return in_maps


def _gather(results):
    out = np.empty((4, COUT, DHW, DHW, DHW), np.float32)
    for c in range(N_CORES):
        b, zh = divmod(c, 2)
        out[b, :, zh * ZS : (zh + 1) * ZS] = results[c]["y"].astype(np.float32)
    return out


def kernel(x, weight):
    from concourse.bass_utils import run_bass_kernel_spmd

    in_maps = _make_in_maps(x, weight)
    nc = _build_program()
    res = run_bass_kernel_spmd(nc, in_maps, list(range(N_CORES)))
    return _gather(res.results)


# revision 10
# speedup vs baseline: 1.2207x; 1.2207x over previous
"""Trainium2 Bass kernel: 3D 'same' convolution (implicit GEMM).

Problem: x (4, 64, 24, 24, 24) f32, weight (1, 128, 1728) f32
         -> out (4, 128, 24, 24, 24) f32  (SAME conv3d, k=3)

Sharding (8 cores): batch (4) x z-halves (2). Each core computes
out[b, :, z0:z0+12] for its (b, zh) shard; no inter-core communication.

Per-core algorithm: 27-tap implicit GEMM in bf16, with taps processed
two-at-a-time per matmul at full K=128 contraction. PE throughput is
set by the single rhs-streaming XBUS (1 column/cycle total), so the
goal is min(total streamed columns) = ceil(27/2) passes x 6912 output
positions. Packing two taps per pass needs the top 64 partitions to
hold the input window *shifted by the tap delta*: three SBUF tiles
carry replicas shifted by +1x, +1y, +1z. 27 taps = 9 x-pairs (tile A)
+ 3 y-pairs (tile B) + 1 z-pair (tile C) + 1 single (K=64).

Measured on HW: a K=128 N=504 bf16 matmul costs ~320 ns (~0.58
ns/col + ~27 ns) regardless of weight-load pattern, vs ~173 ns for
K=64 row-tiled pairs (which stream concurrently) — so 2 taps per
K=128 pass (160 ns/tap) beats the old 27x K=64 scheme (173 ns/tap).
Each weight load is reused across a group of 7 output tiles (7 PSUM
banks in flight, pass-outer loop); redundant legalization-inserted
LDWEIGHTS are elided post-finalize (196 -> ~28, worth ~3 us) and the
weights live in two half tiles so each half's next-iteration reload
hides behind the other half's matmuls.

Output tiles are one z-plane x 21 y-rows x 24 (N=504); the y=21..23
remainder rows are batched across 6 z-planes (N=432) per half-shard.
Group 1 = planes z=0..5 + rem(0..5) reads only padded planes 0..8;
group 2 = planes 6..11 + rem(6..11) reads planes 6..13 — X is loaded
as two overlapping z-chunks so next iteration's chunk-1 DMA overlaps
this iteration's group-2 compute. ACT evacuates PSUM->SBUF with
fp32->bf16 downcast; outputs are stored bf16 and upcast on host
(total quantization error ~3e-3 rel, tolerance 2e-2).
"""

import sys

if "/opt/trn_rl_repo" not in sys.path:
    sys.path.insert(0, "/opt/trn_rl_repo")

import ml_dtypes
import numpy as np

CIN, COUT, K = 64, 128, 3
DHW = 24  # cubic spatial extent
ZS = 12  # z-planes per shard
NP = 14  # padded z-planes per shard window (ZS + 2 halo)
ZA, ZB0, ZB = 9, 6, 8  # chunk1 planes 0..8, chunk2 planes 6..13
PW = 26  # padded y/x extent
N_CORES = 8

# pass table: (tile, tapA, tapB) with tapB = tapA + tile's shift delta;
# tapB None -> single K=64 pass on the tile's bottom half.
# tiles: 0=A (shift +1x), 1=B (+1y), 2=C (+1z)
PASSES = (
    [(0, (dz, dy, 0), (dz, dy, 1)) for dz in range(3) for dy in range(3)]
    + [(1, (dz, 0, 2), (dz, 1, 2)) for dz in range(3)]
    + [(2, (0, 2, 2), (1, 2, 2))]
    + [(0, (2, 2, 2), None)]
)
NPASS = len(PASSES)  # 14


def _elide_redundant_ldweights(nc):
    """Drop legalization-inserted LDWEIGHTS that reload the identical
    weights AP already resident in the PE array (same block, no
    intervening different load). Only sync-free loads are elided."""
    n_drop = 0
    for f in nc.m.functions:
        for b in f.blocks:
            last_key = None
            drop = []
            for inst in b.instructions:
                tn = type(inst).__name__
                if tn == "InstLdweights":
                    key = (str(inst.ins[0]), str(inst.perf_mode), str(inst.is_transpose))
                    si = inst.sync_info
                    clean = si is None or (len(si.on_wait) == 0 and len(si.on_update) == 0)
                    if key == last_key and clean:
                        drop.append(inst)
                    else:
                        last_key = key
            for inst in drop:
                b.instructions.remove(inst)
            n_drop += len(drop)
    return n_drop


def _build_program(loop_n=None, unroll=False):
    """Build the SPMD Bass program (one NeuronCore's view).

    loop_n: if set, wrap the whole body in a hardware For_i loop with
    that many iterations (used by test.py for wall-clock timing).
    unroll: python-unroll the loop instead (for TimelineSim, which
    can't follow register-mode branches).
    """
    import concourse.tile as tile
    from concourse import bacc, mybir

    BF16 = mybir.dt.bfloat16
    F32 = mybir.dt.float32

    nc = bacc.Bacc("TRN2")
    # X inputs: 3 shift-variants x 2 overlapping z-chunks
    xins = []
    for s in "abc":
        x1 = nc.declare_dram_parameter(f"x{s}1", [128, ZA, PW, PW], BF16, isOutput=False)
        x2 = nc.declare_dram_parameter(f"x{s}2", [128, ZB, PW, PW], BF16, isOutput=False)
        xins.append((x1, x2))
    wk_in = nc.declare_dram_parameter("wk", [128, NPASS, 128], BF16, isOutput=False)
    y_out = nc.declare_dram_parameter("y", [128, ZS, DHW, DHW], BF16, isOutput=True)

    with tile.TileContext(nc) as tc:
        with (
            tc.tile_pool(name="xw", bufs=1) as xw_pool,
            tc.tile_pool(name="ps", bufs=8, space="PSUM") as ps_pool,
            tc.tile_pool(name="ob", bufs=4) as ob_pool,
        ):

            def body(_iv=None):
                # two W tiles: each half's reload (next iteration) hides
                # behind the other half's matmuls instead of stalling
                Wa = xw_pool.tile([128, 7, 128], BF16, name="Wa", tag="Wa")
                nc.sync.dma_start(out=Wa[:], in_=wk_in[:, 0:7])
                Wb = xw_pool.tile([128, 7, 128], BF16, name="Wb", tag="Wb")
                nc.sync.dma_start(out=Wb[:], in_=wk_in[:, 7:14])
                XT1, XT2 = [], []
                for s, (x1, x2) in zip("abc", xins):
                    t1 = xw_pool.tile([128, ZA, PW, PW], BF16, name=f"X{s}1", tag=f"X{s}1")
                    nc.sync.dma_start(out=t1[:], in_=x1[:])
                    XT1.append(t1)
                for s, (x1, x2) in zip("abc", xins):
                    t2 = xw_pool.tile([128, ZB, PW, PW], BF16, name=f"X{s}2", tag=f"X{s}2")
                    nc.sync.dma_start(out=t2[:], in_=x2[:])
                    XT2.append(t2)

                # output tiles: ("plane", z) N=504 (21x24, 2D AP)
                #            or ("rem", zoff) N=432 (6x3x24, 3D)
                # group 1 reads padded planes 0..8 (chunk 1), group 2
                # reads planes 6..13 (chunk 2, stored from plane 6).
                groups = [
                    (XT1, 0, [("plane", z) for z in range(6)] + [("rem", 0)]),
                    (XT2, 6, [("plane", z) for z in range(6, 12)] + [("rem", 6)]),
                ]

                def rhs_ap(X, zbase, kind, zi, dz, dy, dx, lo, hi):
                    if kind == "plane":
                        return X[lo:hi, zi - zbase + dz, dy : dy + 21, dx : dx + 24]
                    z0 = zi - zbase + dz
                    return X[lo:hi, z0 : z0 + 6, 21 + dy : 24 + dy, dx : dx + 24]

                for XT, zbase, gtiles in groups:
                    pss = []
                    for kind, zi in gtiles:
                        ps = ps_pool.tile([128, 512], F32, name="ps", tag="ps")
                        pss.append(ps)
                    for j, (ti, ta, tb) in enumerate(PASSES):
                        dz, dy, dx = ta
                        lo, hi = (0, 128) if tb is not None else (0, 64)
                        for (kind, zi), ps in zip(gtiles, pss):
                            n = 504 if kind == "plane" else 432
                            nc.tensor.matmul(
                                ps[:, :n],
                                lhsT=(Wa if j < 7 else Wb)[lo:hi, j % 7, :],
                                rhs=rhs_ap(XT[ti], zbase, kind, zi, dz, dy, dx, lo, hi),
                                start=(j == 0),
                                stop=(j == NPASS - 1),
                                skip_group_check=True,
                            )
                    for (kind, zi), ps in zip(gtiles, pss):
                        n = 504 if kind == "plane" else 432
                        ob = ob_pool.tile([128, 512], BF16, name="ob", tag="ob")
                        nc.scalar.copy(ob[:, :n], ps[:, :n])
                        if kind == "plane":
                            nc.sync.dma_start(out=y_out[:, zi, 0:21, :], in_=ob[:, :n])
                        else:
                            # one DMA per z-plane: keeps each transfer one
                            # contiguous run per partition (descriptor-lean)
                            for j in range(6):
                                nc.sync.dma_start(
                                    out=y_out[:, zi + j, 21:24, :],
                                    in_=ob[:, j * 72 : (j + 1) * 72],
                                )

            if loop_n is not None:
                if unroll:
                    for _k in range(loop_n):
                        body()
                else:
                    with tc.For_i(0, loop_n, 1) as _i:
                        body(_i)
            else:
                body()

    nc.finalize()
    _elide_redundant_ldweights(nc)
    return nc


def _make_in_maps(x, weight):
    w = np.asarray(weight, np.float32).reshape(COUT, CIN, K, K, K)
    wk = np.zeros((128, NPASS, 128), np.float32)
    for j, (_ti, ta, tb) in enumerate(PASSES):
        wk[0:64, j, :] = w[:, :, ta[0], ta[1], ta[2]].T
        if tb is not None:
            wk[64:128, j, :] = w[:, :, tb[0], tb[1], tb[2]].T
    wk = wk.astype(ml_dtypes.bfloat16)

    x = np.asarray(x, np.float32)
    in_maps = []
    for c in range(N_CORES):
        b, zh = divmod(c, 2)
        z0 = zh * ZS
        xpad = np.zeros((CIN, PW, PW, PW), np.float32)
        xpad[:, 1:25, 1:25, 1:25] = x[b]
        win = xpad[:, z0 : z0 + NP]  # (64, 14, 26, 26)

        def repl(shift_axis):
            X = np.zeros((128, NP, PW, PW), np.float32)
            X[0:64] = win
            if shift_axis == 2:  # +1x
                X[64:128, :, :, :-1] = win[:, :, :, 1:]
            elif shift_axis == 1:  # +1y
                X[64:128, :, :-1, :] = win[:, :, 1:, :]
            else:  # +1z
                X[64:128, :-1] = win[:, 1:]
            return X.astype(ml_dtypes.bfloat16)

        m = {"wk": wk}
        for s, ax in (("a", 2), ("b", 1), ("c", 0)):
            X = repl(ax)
            m[f"x{s}1"] = np.ascontiguousarray(X[:, 0:ZA])
            m[f"x{s}2"] = np.ascontiguousarray(X[:, ZB0 : ZB0 + ZB])
        in_maps.append(m)
    return in_maps


def _gather(results):
    out = np.empty((4, COUT, DHW, DHW, DHW), np.float32)
    for c in range(N_CORES):
        b, zh = divmod(c, 2)
        out[b, :, zh * ZS : (zh + 1) * ZS] = results[c]["y"].astype(np.float32)
    return out


def kernel(x, weight):
    from concourse.bass_utils import run_bass_kernel_spmd

    in_maps = _make_in_maps(x, weight)
    nc = _build_program()
    res = run_bass_kernel_spmd(nc, in_maps, list(range(N_CORES)))
    return _gather(res.results)


# revision 11
# speedup vs baseline: 1.2603x; 1.0324x over previous
"""Winograd F(2,3)-y variant: 3D conv = 1D Winograd along y (2-out
tiles, 4 m-terms) x direct 2D conv over (dz,dx) in the m-domain.

vs the x-variant (wino.py): keeping x as the innermost moving dim
preserves 24-element contiguous rhs runs (the x-variant's 12-element
wx runs cost ~35% stream rate), and the output transform's even/odd
y rows are natural slices of the store buffer - no transposed DVE
scatter.

Host: T[cin, k, z, wy, x] = sum_j BT[k,j] xpad[cin, z, 2wy+j, x]
      Gw[k, cout, cin, dz, dx] = sum_dy G[k, dy] w[..., dy, ...]
Device: per k: 9-tap (dz,dx) implicit GEMM over N=(z, wy, x) columns,
      taps paired 2-per-pass at K=128 via +1x / +1z shifted replicas
      (5 passes per k, 20 per output tile). PSUM super-tile = 4 banks
      (one 512-col k-slice each: super n=480 = 2z x 10wy x 24x,
      rem n=288 = 6z x 2wy x 24x). DVE: y[2wy] = m0+m1+m2,
      y[2wy+1] = m1-m2-m3.
"""

import sys

if "/opt/trn_rl_repo" not in sys.path:
    sys.path.insert(0, "/opt/trn_rl_repo")

import ml_dtypes
import numpy as np

CIN, COUT, K = 64, 128, 3
DHW = 24
ZS = 12
NP = 14
ZA, ZB0, ZB = 9, 6, 8  # chunk1 planes 0..8, chunk2 planes 6..13
PW = 26
NWY = 12  # y window pairs
NS = 10  # wy pairs per super tile (rem covers wy 10..11)
NK = 4
N_CORES = 8

BT = np.array([[1, 0, -1, 0], [0, 1, 1, 0], [0, -1, 1, 0], [0, 1, 0, -1]], np.float32)
G = np.array([[1, 0, 0], [0.5, 0.5, 0.5], [0.5, -0.5, 0.5], [0, 0, 1]], np.float32)

# per-k pass table: (tile, tapA, tapB); tile 0 = D (+1x), 1 = E (+1z);
# tapB None -> K=64 single on bottom half. taps are (dz, dx).
KPASSES = (
    [(0, (dz, 0), (dz, 1)) for dz in range(3)]
    + [(1, (0, 2), (1, 2))]
    + [(0, (2, 2), None)]
)
NP_K = len(KPASSES)  # 5


def _elide_redundant_ldweights(nc):
    n_drop = 0
    for f in nc.m.functions:
        for b in f.blocks:
            last_key = None
            drop = []
            for inst in b.instructions:
                if type(inst).__name__ == "InstLdweights":
                    key = (str(inst.ins[0]), str(inst.perf_mode), str(inst.is_transpose))
                    si = inst.sync_info
                    clean = si is None or (len(si.on_wait) == 0 and len(si.on_update) == 0)
                    if key == last_key and clean:
                        drop.append(inst)
                    else:
                        last_key = key
            for inst in drop:
                b.instructions.remove(inst)
            n_drop += len(drop)
    return n_drop


def _build_program(loop_n=None, unroll=False):
    import concourse.tile as tile
    from concourse import bacc, mybir

    BF16 = mybir.dt.bfloat16
    F32 = mybir.dt.float32

    nc = bacc.Bacc("TRN2")
    xins = []
    for s in "de":
        x1 = nc.declare_dram_parameter(f"x{s}1", [128, NK, ZA, NWY, PW], BF16, isOutput=False)
        x2 = nc.declare_dram_parameter(f"x{s}2", [128, NK, ZB, NWY, PW], BF16, isOutput=False)
        xins.append((x1, x2))
    wk_in = nc.declare_dram_parameter("wk", [128, NK * NP_K, 128], BF16, isOutput=False)
    y_out = nc.declare_dram_parameter("y", [128, ZS, DHW, DHW], BF16, isOutput=True)

    with tile.TileContext(nc) as tc:
        with (
            tc.tile_pool(name="xw", bufs=1) as xw_pool,
            tc.tile_pool(name="ps", bufs=2, space="PSUM") as ps_pool,
            tc.tile_pool(name="tmp", bufs=4) as tmp_pool,
            tc.tile_pool(name="ob", bufs=4) as ob_pool,
        ):

            def body(_iv=None):
                Wa = xw_pool.tile([128, 10, 128], BF16, name="Wa", tag="Wa")
                nc.sync.dma_start(out=Wa[:], in_=wk_in[:, 0:10])
                Wb = xw_pool.tile([128, 10, 128], BF16, name="Wb", tag="Wb")
                nc.sync.dma_start(out=Wb[:], in_=wk_in[:, 10:20])
                XT1, XT2 = [], []
                for s, (x1, x2) in zip("de", xins):
                    t1 = xw_pool.tile([128, NK, ZA, NWY, PW], BF16, name=f"X{s}1", tag=f"X{s}1")
                    nc.sync.dma_start(out=t1[:], in_=x1[:])
                    XT1.append(t1)
                for s, (x1, x2) in zip("de", xins):
                    t2 = xw_pool.tile([128, NK, ZB, NWY, PW], BF16, name=f"X{s}2", tag=f"X{s}2")
                    nc.sync.dma_start(out=t2[:], in_=x2[:])
                    XT2.append(t2)

                groups = [
                    (XT1, 0, [("super", 0), ("super", 2)]),
                    (XT1, 0, [("super", 4), ("rem", 0)]),
                    (XT2, 6, [("super", 6), ("super", 8)]),
                    (XT2, 6, [("super", 10), ("rem", 6)]),
                ]

                def rhs_ap(X, zbase, kind, zi, dz, dx, kk, lo, hi):
                    zl = zi - zbase + dz
                    if kind == "super":
                        return X[lo:hi, kk, zl : zl + 2, 0:NS, dx : dx + 24]
                    return X[lo:hi, kk, zl : zl + 6, NS:NWY, dx : dx + 24]

                for XT, zbase, gtiles in groups:
                    pss = [ps_pool.tile([128, 2048], F32, name="ps", tag="ps") for _ in gtiles]
                    for (kind, zi), ps in zip(gtiles, pss):
                        n = 480 if kind == "super" else 288
                        for kk in range(NK):
                            for p, (ti, ta, tb) in enumerate(KPASSES):
                                j = kk * NP_K + p
                                lo, hi = (0, 128) if tb is not None else (0, 64)
                                W = Wa if j < 10 else Wb
                                nc.tensor.matmul(
                                    ps[:, kk * 512 : kk * 512 + n],
                                    lhsT=W[lo:hi, j % 10, :],
                                    rhs=rhs_ap(XT[ti], zbase, kind, zi, ta[0], ta[1], kk, lo, hi),
                                    start=(p == 0),
                                    stop=(p == NP_K - 1),
                                    skip_group_check=True,
                                )
                    for (kind, zi), ps in zip(gtiles, pss):
                        n = 480 if kind == "super" else 288
                        shape = [128, 2, NS, 2, 24] if kind == "super" else [128, 6, 2, 2, 24]
                        ob = ob_pool.tile(shape, BF16, name="ob", tag="ob")
                        t1 = tmp_pool.tile([128, 512], F32, name="t1", tag="t1")
                        t2 = tmp_pool.tile([128, 512], F32, name="t2", tag="t2")
                        c1 = tmp_pool.tile([128, 512], F32, name="c1", tag="c1")
                        c2 = tmp_pool.tile([128, 512], F32, name="c2", tag="c2")
                        m = lambda k: ps[:, k * 512 : k * 512 + n]
                        ye = ob[:, :, :, 0, :]
                        yo = ob[:, :, :, 1, :]
                        # DVE may read at most one PSUM operand per op:
                        # stage m1, m2 into SBUF via ACT first.
                        nc.scalar.copy(c1[:, :n], m(1))
                        nc.scalar.copy(c2[:, :n], m(2))
                        nc.vector.tensor_add(t1[:, :n], m(0), c1[:, :n])
                        nc.vector.tensor_add(ye, t1[:, :n], c2[:, :n])
                        nc.vector.tensor_sub(t2[:, :n], c1[:, :n], c2[:, :n])
                        nc.vector.tensor_sub(yo, t2[:, :n], m(3))
                        if kind == "super":
                            nc.sync.dma_start(out=y_out[:, zi : zi + 2, 0:20, :], in_=ob[:])
                        else:
                            for j6 in range(6):
                                nc.sync.dma_start(
                                    out=y_out[:, zi + j6, 20:24, :], in_=ob[:, j6]
                                )

            if loop_n is not None:
                if unroll:
                    for _k in range(loop_n):
                        body()
                else:
                    with tc.For_i(0, loop_n, 1) as _i:
                        body(_i)
            else:
                body()

    nc.finalize()
    _elide_redundant_ldweights(nc)
    return nc


def _transform_w(weight):
    w = np.asarray(weight, np.float32).reshape(COUT, CIN, K, K, K)
    # Gw[k, cout, cin, dz, dx] = sum_dy G[k, dy] * w[..., dy, ...]
    gw = np.einsum("ky,oczyx->koczx", G, w)
    wk = np.zeros((128, NK * NP_K, 128), np.float32)
    for kk in range(NK):
        for p, (_ti, ta, tb) in enumerate(KPASSES):
            j = kk * NP_K + p
            wk[0:64, j, :] = gw[kk, :, :, ta[0], ta[1]].T
            if tb is not None:
                wk[64:128, j, :] = gw[kk, :, :, tb[0], tb[1]].T
    return wk.astype(ml_dtypes.bfloat16)


def _make_in_maps(x, weight):
    wk = _transform_w(weight)
    x = np.asarray(x, np.float32)
    in_maps = []
    for c in range(N_CORES):
        b, zh = divmod(c, 2)
        z0 = zh * ZS
        xpad = np.zeros((CIN, PW, PW, PW), np.float32)
        xpad[:, 1:25, 1:25, 1:25] = x[b]
        win = xpad[:, z0 : z0 + NP]  # (64, 14, 26, 26)
        # T[cin, k, z, wy, x] = sum_j BT[k, j] win[cin, z, 2wy+j, x]
        wmat = np.lib.stride_tricks.sliding_window_view(win, 4, axis=2)[:, :, ::2]
        # wmat: (64, 14, 12, 26, 4)
        T = np.einsum("kj,czwxj->ckzwx", BT, wmat)

        def repl(shift_axis):
            X = np.zeros((128, NK, NP, NWY, PW), np.float32)
            X[0:64] = T
            if shift_axis == 3:  # +1x
                X[64:128, :, :, :, :-1] = T[:, :, :, :, 1:]
            else:  # +1z
                X[64:128, :, :-1] = T[:, :, 1:]
            return X.astype(ml_dtypes.bfloat16)

        m = {"wk": wk}
        for s, ax in (("d", 3), ("e", 0)):
            X = repl(ax)
            m[f"x{s}1"] = np.ascontiguousarray(X[:, :, 0:ZA])
            m[f"x{s}2"] = np.ascontiguousarray(X[:, :, ZB0 : ZB0 + ZB])
        in_maps.append(m)
    return in_maps


def _gather(results):
    out = np.empty((4, COUT, DHW, DHW, DHW), np.float32)
    for c in range(N_CORES):
        b, zh = divmod(c, 2)
        out[b, :, zh * ZS : (zh + 1) * ZS] = results[c]["y"].astype(np.float32)
    return out


def kernel(x, weight):
    from concourse.bass_utils import run_bass_kernel_spmd

    in_maps = _make_in_maps(x, weight)
    nc = _build_program()
    res = run_bass_kernel_spmd(nc, in_maps, list(range(N_CORES)))
    return _gather(res.results)


def _emulate_core(m, core=0):
    XT = {}
    for s in "de":
        XT[s] = (np.asarray(m[f"x{s}1"], np.float32), np.asarray(m[f"x{s}2"], np.float32))
    WK = np.asarray(m["wk"], np.float32)
    y = np.zeros((128, ZS, DHW, DHW), np.float32)
    groups = [
        (0, 0, [("super", 0), ("super", 2)]),
        (0, 0, [("super", 4), ("rem", 0)]),
        (1, 6, [("super", 6), ("super", 8)]),
        (1, 6, [("super", 10), ("rem", 6)]),
    ]
    for ci, zbase, gtiles in groups:
        for kind, zi in gtiles:
            n = 480 if kind == "super" else 288
            ps = np.zeros((128, NK, n), np.float32)
            for kk in range(NK):
                for p, (ti, ta, tb) in enumerate(KPASSES):
                    j = kk * NP_K + p
                    lo, hi = (0, 128) if tb is not None else (0, 64)
                    X = XT["de"[ti]][ci]
                    zl = zi - zbase + ta[0]
                    dx = ta[1]
                    if kind == "super":
                        r = X[lo:hi, kk, zl : zl + 2, 0:NS, dx : dx + 24]
                    else:
                        r = X[lo:hi, kk, zl : zl + 6, NS:NWY, dx : dx + 24]
                    ps[:, kk] += WK[lo:hi, j, :].T @ r.reshape(hi - lo, -1)
            ye = (ps[:, 0] + ps[:, 1] + ps[:, 2]).astype(ml_dtypes.bfloat16).astype(np.float32)
            yo = (ps[:, 1] - ps[:, 2] - ps[:, 3]).astype(ml_dtypes.bfloat16).astype(np.float32)
            if kind == "super":
                yi = np.stack([ye.reshape(128, 2, NS, 24), yo.reshape(128, 2, NS, 24)], 3)
                y[:, zi : zi + 2, 0:20, :] = yi.reshape(128, 2, 20, 24)
            else:
                yi = np.stack([ye.reshape(128, 6, 2, 24), yo.reshape(128, 6, 2, 24)], 3)
                y[:, zi : zi + 6, 20:24, :] = yi.reshape(128, 6, 4, 24)
    return y


if __name__ == "__main__":
    import jax

    sys.path.insert(0, "/root/problem")
    import reference

    cpu = jax.devices("cpu")[0]
    with jax.default_device(cpu):
        inputs = {k: np.asarray(v) for k, v in reference.setup_inputs().items()}
        expected = np.asarray(
            reference.reference(**{k: jax.device_put(v, cpu) for k, v in inputs.items()})
        )
    in_maps = _make_in_maps(inputs["x"], inputs["weight"])
    y = _emulate_core(in_maps[0])
    exp = expected[0][:, 0:ZS]
    err = np.linalg.norm(y - exp) / np.linalg.norm(exp)
    print("emulated core0 rel err:", err)
